# revision 46
# baseline (speedup 1.0000x reference)
"""TRN2 Bass kernel for nn_Attention_87308095193383.

Sharding: 8 cores = (batch b in 0..3) x (query-half h in 0..1).
Host permutes N columns per core so "my queries" are columns 0:NQ.

Per core:
  A. conv1/conv2 (f32r) + GroupNorm (bn_stats on bf16 raw + manual combine,
     group reduce/broadcast via tiny indicator matmuls, affine on Pool).
  B. pe_attn^T = sigmoid(p2^T p1) via fp8 DoubleRow matmuls -> pa bf16.
  C. qkv (f32r). k/q written as fp8 quadrant tiles [32part,2dh,2hg,n] via
     host-permuted weight columns; v bf16 with interleaved ones column.
  E. qk via fp8 DoubleRow (2x), gating on DVE (some tiles via Act-copy +
     Pool-mult), exp on Act in 4096-wide ops, attn@v TRANSPOSED (out [q,65])
     in bf16 with 4 query-tile accumulators packed per PSUM bank.
  F. division via per-qt tensor_scalar, DMA-transpose o^T -> o_c, proj with
     bf16 moving operand, bias folded into Act Identity copy.
"""
import numpy as np
import ml_dtypes

import concourse.bass as bass
import concourse.mybir as mybir
import concourse.tile as tile
from concourse import bacc
from concourse.bass_utils import run_bass_kernel_spmd

F32R = mybir.dt.float32r
F32 = mybir.dt.float32
BF16 = mybir.dt.bfloat16
FP8 = mybir.dt.float8e4
AF = mybir.ActivationFunctionType
ALU = mybir.AluOpType
DR = mybir.MatmulPerfMode.DoubleRow

N_CORES = 8
C = 512          # channels
CT = C // 128    # 4 c-tiles
N = 2048         # sequence length
NT = N // 128    # 16 m-tiles
NQ = 1024        # queries per core
H = 8            # heads
D = 64           # head dim
SCALE = D ** -0.5
EPS = 1e-5

DEBUG = False
PEATTN_FP8 = False     # pe_attn matmul in fp8 DoubleRow
QK_FP8 = True         # q@k in fp8 DoubleRow
POOL_GATE = (0,)      # which mtp in 0..3 gate via Act-copy + Pool-mult

PA_DT = FP8 if PEATTN_FP8 else BF16


def build():
    nc = bacc.Bacc("TRN2", target_bir_lowering=False, debug=False,
                   num_devices=N_CORES)

    def din(name, shape, dt=F32R):
        return nc.dram_tensor(name, shape, dt, kind="ExternalInput").ap()

    peT = din("peT", [C, N], BF16)
    xT = din("xT", [C, N], BF16)
    cw1 = din("cw1", [C, C], BF16)  # conv1_w.T  [c_in, o]
    cw2 = din("cw2", [C, C], BF16)
    qw = din("qw", [C, 3 * C], BF16)  # qkv_w.T, q/k sections col-permuted
    pw = din("pw", [C, C], BF16)    # proj_w.T (bf16: moving o_c is bf16)
    cb1 = din("cb1", [C], F32)
    cb2 = din("cb2", [C], F32)
    gn1g = din("gn1g", [C], F32)
    gn1b = din("gn1b", [C], F32)
    gn2g = din("gn2g", [C], F32)
    gn2b = din("gn2b", [C], F32)
    pb = din("pb", [C], F32)
    gmask_in = din("gmask", [128, 2], F32)
    gmaskT_in = din("gmaskT", [2, 128], F32)
    vones_in = din("vones", [128, NT * H], BF16)
    ident_in = din("ident", [128, 128], BF16)
    outT = nc.dram_tensor("outT", [C, NQ], F32, kind="ExternalOutput").ap()
    dbg = {}
    if DEBUG:
        def dout(name, shape, dt):
            dbg[name] = nc.dram_tensor("dbg_" + name, shape, dt,
                                       kind="ExternalOutput").ap()
        dout("p1r", [128, CT, N], BF16)
        dout("p2r", [128, CT, N], BF16)
        dout("p1", [128, CT, NQ], PA_DT)
        dout("p2", [128, CT, N], PA_DT)
        dout("pa", [128, NT, NQ], BF16)
        dout("k8", [128, 2, 2, N], FP8 if QK_FP8 else BF16)
        dout("q8", [128, 2, 2, NQ], FP8 if QK_FP8 else BF16)
        dout("v", [128, NT, H, D + 1], BF16)
        dout("oT", [128, 8, H, D], BF16)
        dout("oc", [128, 8, CT, 128], BF16)

    with tile.TileContext(nc) as tc:
        _body(nc, tc, peT, xT, cw1, cw2, qw, pw, cb1, cb2,
              gn1g, gn1b, gn2g, gn2b, pb, gmask_in, gmaskT_in, vones_in,
              ident_in, outT, dbg)
    nc.compile()
    return nc


def _body(nc, tc, peT, xT, cw1, cw2, qw, pw, cb1, cb2,
          gn1g, gn1b, gn2g, gn2b, pb, gmask_in, gmaskT_in, vones_in,
          ident_in, outT, dbg):
    from contextlib import ExitStack
    ctx = ExitStack()
    with ctx:
        consts = ctx.enter_context(tc.tile_pool(name="consts", bufs=1))
        work = ctx.enter_context(tc.tile_pool(name="work", bufs=3))


        # ---- persistent activations
        pa_pool = ctx.enter_context(tc.tile_pool(name="pa", bufs=1))
        pa = pa_pool.tile([128, NT, NQ], BF16)     # sigmoid gate ^T tiles

        p12_pool = ctx.enter_context(tc.tile_pool(name="p12", bufs=1))
        kqv_pool = ctx.enter_context(tc.tile_pool(name="kqv", bufs=1))
        x_pool = ctx.enter_context(tc.tile_pool(name="x_pool", bufs=1))
        qw_pool = ctx.enter_context(tc.tile_pool(name="qw_pool", bufs=1))
        ps_main = ctx.enter_context(tc.tile_pool(name="ps_main", bufs=3,
                                                 space="PSUM"))

        # ================= stage A: conv + groupnorm =================
        ab = ExitStack()
        pe_pool = ab.enter_context(tc.tile_pool(name="pe_pool", bufs=1))
        cw_pool = ab.enter_context(tc.tile_pool(name="cw_pool", bufs=1))
        praw_pool = ab.enter_context(tc.tile_pool(name="praw", bufs=1))
        ps_gn = ab.enter_context(tc.tile_pool(name="ps_gn", bufs=1,
                                              space="PSUM"))

        pe_sb = pe_pool.tile([128, CT, N], BF16)
        pe_r = peT.rearrange("(t p) n -> p t n", p=128)
        cw1_sb = cw_pool.tile([128, CT, C], BF16)
        cw1_r = cw1.rearrange("(t p) o -> p t o", p=128)
        cw2_sb = cw_pool.tile([128, CT, C], BF16)
        nc.gpsimd.dma_start(cw1_sb[:, :, 0:256], cw1_r[:, :, 0:256])
        nc.sync.dma_start(pe_sb[:, :, 0:512], pe_r[:, :, 0:512])
        nc.gpsimd.dma_start(cw1_sb[:, :, 256:512], cw1_r[:, :, 256:512])
        nc.scalar.dma_start(pe_sb[:, :, 512:1024], pe_r[:, :, 512:1024])
        nc.sync.dma_start(pe_sb[:, :, 1024:1536], pe_r[:, :, 1024:1536])
        nc.scalar.dma_start(cw2_sb, cw2.rearrange("(t p) o -> p t o", p=128))
        nc.gpsimd.dma_start(pe_sb[:, :, 1536:2048], pe_r[:, :, 1536:2048])
        # x/qw/pw loads start now, drain behind the conv inputs
        x_sb = x_pool.tile([128, CT, N], BF16)
        x_r = xT.rearrange("(t p) n -> p t n", p=128)
        qw_sb = qw_pool.tile([128, CT, 3 * C], BF16)
        qw_r = qw.rearrange("(t p) o -> p t o", p=128)
        for cc, eng in enumerate((nc.sync, nc.scalar, nc.gpsimd, nc.sync)):
            eng.dma_start(x_sb[:, :, cc * 512:(cc + 1) * 512],
                          x_r[:, :, cc * 512:(cc + 1) * 512])
        for sec, eng in ((2, nc.sync), (1, nc.scalar), (0, nc.gpsimd)):
            eng.dma_start(qw_sb[:, :, sec * C:(sec + 1) * C],
                          qw_r[:, :, sec * C:(sec + 1) * C])
        # ---- constants (emitted on scalar queue to keep sync free for pe)
        gmask = consts.tile([128, 2], F32)
        nc.gpsimd.dma_start(gmask, gmask_in)
        gmaskT = consts.tile([2, 128], F32)
        nc.gpsimd.dma_start(gmaskT, gmaskT_in)
        epst = consts.tile([128, 1], F32)
        nc.vector.memset(epst, EPS)
        bias1 = consts.tile([128, CT], F32)
        nc.sync.dma_start(bias1, cb1.rearrange("(t p) -> p t", p=128))
        bias2 = consts.tile([128, CT], F32)
        nc.sync.dma_start(bias2, cb2.rearrange("(t p) -> p t", p=128))
        g1g = consts.tile([128, CT], F32)
        nc.sync.dma_start(g1g, gn1g.rearrange("(t p) -> p t", p=128))
        g1b = consts.tile([128, CT], F32)
        nc.sync.dma_start(g1b, gn1b.rearrange("(t p) -> p t", p=128))
        g2g = consts.tile([128, CT], F32)
        nc.sync.dma_start(g2g, gn2g.rearrange("(t p) -> p t", p=128))
        g2b = consts.tile([128, CT], F32)
        nc.sync.dma_start(g2b, gn2b.rearrange("(t p) -> p t", p=128))
        pbias = consts.tile([128, CT], F32)
        nc.sync.dma_start(pbias, pb.rearrange("(t p) -> p t", p=128))
        ident = consts.tile([128, 128], BF16)
        nc.sync.dma_start(ident, ident_in)

        p1_raw = praw_pool.tile([128, CT, N], BF16)
        p2_raw = praw_pool.tile([128, CT, N], BF16)
        p1_sb = p12_pool.tile([128, CT, NQ], PA_DT)
        p2_sb = p12_pool.tile([128, CT, N], PA_DT)

        for cwsb, cbt, gg, gb, raw, dst, keep in (
                (cw1_sb, bias1, g1g, g1b, p1_raw, p1_sb, NQ),
                (cw2_sb, bias2, g2g, g2b, p2_raw, p2_sb, N)):
            # conv matmuls + biased copies
            for nc2 in range(2):
                for ot in range(CT):
                    T = ps_main.tile([128, 2, 512], F32, tag="mm")
                    for half in range(2):
                        nch = nc2 * 2 + half
                        for ct in range(CT):
                            nc.tensor.matmul(
                                T[:, half], cwsb[:, ct, ot * 128:(ot + 1) * 128],
                                pe_sb[:, ct, nch * 512:(nch + 1) * 512],
                                start=(ct == 0), stop=(ct == CT - 1))
                    nc.scalar.activation(
                        raw[:, ot, nc2 * 1024:(nc2 + 1) * 1024], T,
                        AF.Identity, bias=cbt[:, ot:ot + 1])
            # stats on biased bf16 raw
            st = work.tile([128, CT, 4, 6], F32, tag="gnstats")
            for ot in range(CT):
                for ch in range(4):
                    nc.vector.bn_stats(st[:, ot, ch],
                                       raw[:, ot, ch * 512:(ch + 1) * 512])
            # combine chunk stats -> per-channel [mean, E[x^2]]
            A = work.tile([128, CT, 4], F32, tag="gnA")
            nc.vector.tensor_add(A, st[:, :, :, 1], st[:, :, :, 4])
            G2 = work.tile([128, CT, 4], F32, tag="gnG2")
            nc.vector.tensor_add(G2, st[:, :, :, 2], st[:, :, :, 5])
            P = work.tile([128, CT, 4], F32, tag="gnP")
            nc.vector.tensor_mul(P, st[:, :, :, 1], st[:, :, :, 1])
            Q = work.tile([128, CT, 4], F32, tag="gnQ")
            nc.vector.tensor_mul(Q, st[:, :, :, 4], st[:, :, :, 4])
            nc.vector.tensor_add(P, P, Q)
            nc.vector.tensor_scalar_mul(P, P, 256.0)
            nc.vector.tensor_add(P, P, G2)           # h per chunk
            SA = work.tile([128, CT, 2], F32, tag="gnSA")
            nc.vector.tensor_add(SA, A[:, :, 0:2], A[:, :, 2:4])
            SH = work.tile([128, CT, 2], F32, tag="gnSH")
            nc.vector.tensor_add(SH, P[:, :, 0:2], P[:, :, 2:4])
            stack = work.tile([128, 2, CT], F32, tag="gnstack")
            La = work.tile([128, CT], F32, tag="gnL0")
            nc.vector.tensor_add(La, SA[:, :, 0], SA[:, :, 1])
            nc.vector.tensor_scalar_mul(stack[:, 0], La, 1.0 / 8.0)
            nc.vector.tensor_add(La, SH[:, :, 0], SH[:, :, 1])
            nc.vector.tensor_scalar_mul(stack[:, 1], La, 1.0 / 2048.0)
            # group sums over 64-partition halves
            gsp = ps_gn.tile([2, 2, CT], F32, tag="gn")
            nc.tensor.matmul(gsp, gmask, stack.rearrange("p a t -> p (a t)"),
                             start=True, stop=True)
            gss = work.tile([2, 2, CT], F32, tag="gss")
            nc.vector.tensor_copy(gss, gsp)
            msr = work.tile([2, 2, CT], F32, tag="gmsr")  # [mean, rstd]
            nc.vector.tensor_scalar_mul(msr[:, 0], gss[:, 0], 1.0 / 64.0)
            vt = work.tile([2, 2, CT], F32, tag="gvt")
            nc.vector.tensor_scalar_mul(vt[:, 0], gss[:, 1], 1.0 / 64.0)
            nc.vector.tensor_mul(vt[:, 1], msr[:, 0], msr[:, 0])
            nc.vector.tensor_sub(vt[:, 0], vt[:, 0], vt[:, 1])
            nc.scalar.activation(vt[:, 0], vt[:, 0], AF.Sqrt, bias=epst[0:2])
            nc.vector.reciprocal(msr[:, 1], vt[:, 0])
            # broadcast [mean, rstd] to 128 partitions
            bcp = ps_gn.tile([128, 2, CT], F32, tag="gnb")
            nc.tensor.matmul(bcp, gmaskT, msr.rearrange("p a t -> p (a t)"),
                             start=True, stop=True)
            bcst = work.tile([128, 2, CT], F32, tag="gbc")
            nc.vector.tensor_copy(bcst, bcp)
            sc = work.tile([128, 2, CT], F32, tag="gsc")
            nc.vector.tensor_mul(sc[:, 0], bcst[:, 1], gg)
            nc.vector.tensor_mul(sc[:, 1], bcst[:, 0], sc[:, 0])
            nc.vector.tensor_sub(sc[:, 1], gb, sc[:, 1])
            for ot in range(CT):
                eng = nc.gpsimd if ot < 2 else nc.vector
                eng.tensor_scalar(
                    dst[:, ot, 0:keep], raw[:, ot, 0:keep],
                    sc[:, 0, ot:ot + 1], sc[:, 1, ot:ot + 1],
                    op0=ALU.mult, op1=ALU.add)

        if dbg:
            nc.sync.dma_start(dbg["p1r"], p1_raw)
            nc.sync.dma_start(dbg["p2r"], p2_raw)
            nc.sync.dma_start(dbg["p1"], p1_sb)
            nc.sync.dma_start(dbg["p2"], p2_sb)
        ab.close()

        # ================= stage B: pa = sigmoid(p2^T p1) =================
        # ================= stage C: qkv =================
        KQ_DT = FP8 if QK_FP8 else BF16
        k8 = kqv_pool.tile([128, 2, 2, N], KQ_DT)    # [quad-part, dh, hg, m]
        q8 = kqv_pool.tile([128, 2, 2, NQ], KQ_DT)
        v_sb = kqv_pool.tile([128, NT, H, D + 1], BF16)
        nc.sync.dma_start(
            v_sb[:, :, :, D:D + 1].rearrange("p t o u -> p (t o u)"),
            vones_in)

        def emit_v(nts=range(NT)):
            for nt in nts:
                T = ps_main.tile([128, 2, 512], F32, tag="mm")
                for ct in range(CT):
                    nc.tensor.matmul(
                        T[:, 0], x_sb[:, ct, nt * 128:(nt + 1) * 128],
                        qw_sb[:, ct, 2 * C:2 * C + 512],
                        start=(ct == 0), stop=(ct == CT - 1))
                nc.scalar.copy(v_sb[:, nt, :, 0:D],
                               T[:, 0].rearrange("p (h d) -> p h d", h=H))

        def emit_kq(hg, dhs=(0, 1)):
            for dh in dhs:
                ti = dh * 2 + hg
                for nch in range(4):
                    T = ps_main.tile([128, 2, 512], F32, tag="mm")
                    for ct in range(CT):
                        nc.tensor.matmul(
                            T[:, 0],
                            qw_sb[:, ct, C + ti * 128:C + (ti + 1) * 128],
                            x_sb[:, ct, nch * 512:(nch + 1) * 512],
                            start=(ct == 0), stop=(ct == CT - 1))
                    keng = nc.vector if nch % 2 else nc.scalar
                    if keng is nc.vector:
                        keng.tensor_copy(
                            k8[:, dh, hg, nch * 512:(nch + 1) * 512], T[:, 0])
                    else:
                        keng.copy(
                            k8[:, dh, hg, nch * 512:(nch + 1) * 512], T[:, 0])
                for nch in range(2):
                    T = ps_main.tile([128, 2, 512], F32, tag="mm")
                    for ct in range(CT):
                        nc.tensor.matmul(
                            T[:, 0], qw_sb[:, ct, ti * 128:(ti + 1) * 128],
                            x_sb[:, ct, nch * 512:(nch + 1) * 512],
                            start=(ct == 0), stop=(ct == CT - 1))
                    if nch % 2:
                        nc.vector.tensor_copy(
                            q8[:, dh, hg, nch * 512:(nch + 1) * 512], T[:, 0])
                    else:
                        nc.scalar.copy(
                            q8[:, dh, hg, nch * 512:(nch + 1) * 512], T[:, 0])

        def emit_peattn(nqch, mtps=range(NT // 2)):
            # pa holds tanh(z/2); sigmoid = (1 + tanh)/2, the 1/2 folded
            # into the exp scale downstream.
            for mtp in mtps:
                T = ps_main.tile([128, 2, 512], F32, tag="mm")
                for j in range(2):
                    mt = 2 * mtp + j
                    if PEATTN_FP8:
                        for cp in range(2):
                            nc.tensor.matmul(
                                T[:, j],
                                p2_sb[:, 2 * cp:2 * cp + 2, mt * 128:(mt + 1) * 128],
                                p1_sb[:, 2 * cp:2 * cp + 2, nqch * 512:(nqch + 1) * 512],
                                start=(cp == 0), stop=(cp == 1), perf_mode=DR)
                    else:
                        for ct in range(CT):
                            nc.tensor.matmul(
                                T[:, j], p2_sb[:, ct, mt * 128:(mt + 1) * 128],
                                p1_sb[:, ct, nqch * 512:(nqch + 1) * 512],
                                start=(ct == 0), stop=(ct == CT - 1))
                pa_sl = pa[:, 2 * mtp:2 * mtp + 2,
                           nqch * 512:(nqch + 1) * 512]
                nc.scalar.activation(pa_sl, T, AF.Tanh, scale=0.5)
                nc.gpsimd.tensor_scalar_add(pa_sl, pa_sl, 1.0)

        if dbg:
            nc.sync.dma_start(dbg["pa"], pa)
            nc.sync.dma_start(dbg["k8"], k8)
            nc.sync.dma_start(dbg["q8"], q8)
            nc.sync.dma_start(dbg["v"], v_sb)

        # ================= stage E: attention =================
        e_pools = ExitStack()
        ps_av = e_pools.enter_context(tc.tile_pool(name="ps_av", bufs=2,
                                                   space="PSUM"))
        te_pool = e_pools.enter_context(tc.tile_pool(name="te", bufs=3))
        cp_pool = e_pools.enter_context(tc.tile_pool(name="cp", bufs=3))
        o_pool = e_pools.enter_context(tc.tile_pool(name="o", bufs=1))
        pw_pool = e_pools.enter_context(tc.tile_pool(name="pw", bufs=1))
        fin_pool = e_pools.enter_context(tc.tile_pool(name="fin", bufs=2))

        pw_sb = pw_pool.tile([128, CT, C], BF16)
        nc.sync.dma_start(pw_sb, pw.rearrange("(t p) o -> p t o", p=128))
        oT = o_pool.tile([128, 8, H, D], BF16)       # [q-part, qtg, h, d]
        o_c = o_pool.tile([128, 8, CT, 128], BF16)   # [q-part, qtg, ct, q128]
        outT_r = outT.rearrange("(t p) n -> p t n", p=128)

        def emit_qk_gate_exp(nq, h, half, par=0):
            """qk (fp8 DR), gating, exp for 8 m-tiles. Returns e2 tile."""
            hm4, hg = h % 4, h // 4
            p0 = hm4 * 32
            t2 = te_pool.tile([128, 8, 512], BF16, tag="t2")
            for mtp in range(4):
                S = ps_main.tile([128, 2, 512], F32, tag="mm")
                for j in range(2):
                    mt = half * 8 + 2 * mtp + j
                    if QK_FP8:
                        nc.tensor.matmul(
                            S[:, j],
                            k8[p0:p0 + 32, :, hg, mt * 128:(mt + 1) * 128],
                            q8[p0:p0 + 32, :, hg, nq * 512:(nq + 1) * 512],
                            start=True, stop=True, perf_mode=DR,
                            tile_position=(p0, 0))
                    else:
                        for dh in range(2):
                            nc.tensor.matmul(
                                S[:, j],
                                k8[p0:p0 + 32, dh, hg, mt * 128:(mt + 1) * 128],
                                q8[p0:p0 + 32, dh, hg, nq * 512:(nq + 1) * 512],
                                start=(dh == 0), stop=(dh == 1),
                                tile_position=(p0, 0))
                pa_sl = pa[:, half * 8 + 2 * mtp:half * 8 + 2 * mtp + 2,
                           nq * 512:(nq + 1) * 512]
                if mtp in POOL_GATE:
                    sc_ = cp_pool.tile([128, 2, 512], BF16, tag="sc")
                    if par:
                        nc.vector.tensor_copy(sc_, S)
                    else:
                        nc.scalar.copy(sc_, S)
                    nc.gpsimd.tensor_mul(t2[:, 2 * mtp:2 * mtp + 2], sc_, pa_sl)
                else:
                    nc.vector.tensor_mul(t2[:, 2 * mtp:2 * mtp + 2], S, pa_sl)
            e2 = te_pool.tile([128, 8, 512], BF16, tag="e2")
            nc.scalar.activation(e2[:, 0:4], t2[:, 0:4], AF.Exp,
                                 scale=SCALE * 0.5)
            nc.scalar.activation(e2[:, 4:8], t2[:, 4:8], AF.Exp,
                                 scale=SCALE * 0.5)
            return e2

        def emit_av(nq, h, half, e2, uacc):
            for mtp8 in range(8):
                mt = half * 8 + mtp8
                for qt in range(4):
                    nc.tensor.matmul(
                        uacc[:, qt], e2[:, mtp8, qt * 128:(qt + 1) * 128],
                        v_sb[:, mt, h, :],
                        start=(mt == 0 and qt == 0),
                        stop=(mt == NT - 1 and qt == 3),
                        skip_group_check=True)

        def emit_div(nq, h, uacc):
            rec = work.tile([128, 4, 1], F32, tag="rec")
            nc.vector.reciprocal(rec, uacc[:, :, D:D + 1])
            for qt in range(4):
                nc.vector.tensor_scalar_mul(
                    oT[:, nq * 4 + qt, h, :], uacc[:, qt, 0:D],
                    rec[:, qt])

        def emit_proj_step(nq, step):
            if step < 4:
                qtg = nq * 4 + step
                src_q = oT[:, qtg].rearrange("p h d -> p (h d)")
                TRf = ps_main.tile([128, 2, 512], F32, tag="mm")
                TR = TRf[:, 0, 0:256].bitcast(BF16).rearrange(
                    "p (t n) -> p t n", t=4)
                for tb in range(4):
                    nc.tensor.matmul(
                        TR[:, tb], src_q[:, tb * 128:(tb + 1) * 128], ident,
                        is_transpose=True, start=(tb == 0), stop=(tb == 3),
                        skip_group_check=True)
                nc.scalar.copy(o_c[:, qtg], TR)
            else:
                ot = step - 4
                Pf = ps_main.tile([128, 2, 512], F32, tag="mm")
                P = Pf[:, 0]
                for ct in range(CT):
                    nc.tensor.matmul(
                        P, pw_sb[:, ct, ot * 128:(ot + 1) * 128],
                        o_c[:, nq * 4:(nq + 1) * 4, ct, :],
                        start=(ct == 0), stop=(ct == CT - 1))
                fin = fin_pool.tile([128, 512], F32, tag="fin")
                nc.scalar.activation(fin, P, AF.Identity,
                                     bias=pbias[:, ot:ot + 1])
                nc.sync.dma_start(outT_r[:, ot, nq * 512:(nq + 1) * 512], fin)

        def emit_proj(nq):
            for step in range(8):
                emit_proj_step(nq, step)

        # software-pipelined emission: av lags one block; div lags two;
        # proj lags three. Keeps each engine queue free of long waits.
        blocks = [(nq, h, half) for nq in range(2) for h in range(H)
                  for half in range(2)]
        av_pending = None
        div_sched = {}
        proj_sched = {}
        uacc_cur = None

        def flush(i):
            if i in div_sched:
                emit_div(*div_sched.pop(i))
            if i in proj_sched:
                nqp, step = proj_sched.pop(i)
                emit_proj_step(nqp, step)

        emit_v()
        emit_peattn(0)
        emit_kq(0)

        hooks = {8: lambda: (emit_peattn(1), emit_kq(1))}
        for i, (nq, h, half) in enumerate(blocks):
            if i in hooks:
                hooks.pop(i)()
            if half == 0:
                uacc_cur = ps_av.tile([128, 4, D + 1], F32, tag="av")
            e2 = emit_qk_gate_exp(nq, h, half, par=(i % 3 == 2))
            if av_pending is not None:
                emit_av(*av_pending)
            av_pending = (nq, h, half, e2, uacc_cur)
            if half == 1:
                div_sched[i + 2] = (nq, h, uacc_cur)
                if h == H - 1:
                    for step in range(8):
                        proj_sched[i + 3 + step] = (nq, step)
            flush(i)
        emit_av(*av_pending)
        for j in sorted(set(list(div_sched) + list(proj_sched))):
            flush(j)
        if dbg:
            nc.sync.dma_start(dbg["oT"], oT)
            nc.sync.dma_start(dbg["oc"], o_c)
        e_pools.close()


_NC_CACHE = {}


def _get_nc():
    if "nc" not in _NC_CACHE:
        _NC_CACHE["nc"] = build()
    return _NC_CACHE["nc"]


def _qk_perm():
    """Column permutation for q/k sections of qkv_w so PSUM partition
    layout matches the fp8 quadrant tiles [32, dh, hg]."""
    perm = np.empty(C, np.int64)
    for ti in range(4):
        dh, hg = ti // 2, ti % 2
        for p in range(128):
            hm4, p32 = p // 32, p % 32
            h = hg * 4 + hm4
            d = dh * 32 + p32
            perm[ti * 128 + p] = h * D + d
    return perm


def make_in_maps(x, pe, qkv_w, proj_w, proj_b, conv1_w, conv1_b, gn1_g, gn1_b,
                 conv2_w, conv2_b, gn2_g, gn2_b):
    f = np.float32
    qwt = np.asarray(qkv_w, f).T          # [c_in, 3C]
    perm = _qk_perm()
    qw_host = np.concatenate([qwt[:, 0:C][:, perm],
                              qwt[:, C:2 * C][:, perm],
                              qwt[:, 2 * C:3 * C]], axis=1)
    shared = {
        "cw1": np.ascontiguousarray(np.asarray(conv1_w, f).T).astype(ml_dtypes.bfloat16),
        "cw2": np.ascontiguousarray(np.asarray(conv2_w, f).T).astype(ml_dtypes.bfloat16),
        "qw": np.ascontiguousarray(qw_host).astype(ml_dtypes.bfloat16),
        "pw": np.ascontiguousarray(np.asarray(proj_w, f).T).astype(ml_dtypes.bfloat16),
        "cb1": np.asarray(conv1_b, f),
        "cb2": np.asarray(conv2_b, f),
        "gn1g": np.asarray(gn1_g, f),
        "gn1b": np.asarray(gn1_b, f),
        "gn2g": np.asarray(gn2_g, f),
        "gn2b": np.asarray(gn2_b, f),
        "pb": np.asarray(proj_b, f),
        "gmask": np.repeat(np.eye(2, dtype=f), 64, axis=0),
        "gmaskT": np.ascontiguousarray(
            np.repeat(np.eye(2, dtype=f), 64, axis=0).T),
        "vones": np.ones((128, NT * H), np.float32).astype(ml_dtypes.bfloat16),
        "ident": np.eye(128, dtype=np.float32).astype(ml_dtypes.bfloat16),
    }
    in_maps = []
    for c in range(N_CORES):
        b, h = c // 2, c % 2
        xT = np.asarray(x[b], f).T
        peT = np.asarray(pe[b], f).T
        if h == 1:
            xT = np.concatenate([xT[:, NQ:], xT[:, :NQ]], axis=1)
            peT = np.concatenate([peT[:, NQ:], peT[:, :NQ]], axis=1)
        m = dict(shared)
        m["xT"] = np.ascontiguousarray(xT).astype(ml_dtypes.bfloat16)
        m["peT"] = np.ascontiguousarray(peT).astype(ml_dtypes.bfloat16)
        in_maps.append(m)
    return in_maps


def assemble_out(results):
    B = N_CORES // 2
    out = np.empty((B, N, C), np.float32)
    for c in range(N_CORES):
        b, h = c // 2, c % 2
        out[b, h * NQ:(h + 1) * NQ, :] = results[c]["outT"].T
    return out


def kernel(**inputs):
    nc = _get_nc()
    in_maps = make_in_maps(**inputs)
    r = run_bass_kernel_spmd(nc, in_maps, core_ids=list(range(N_CORES)))
    return assemble_out(r.results)


if __name__ == "__main__":
    nc = build()
    print("build+compile OK")


# revision 47
# speedup vs baseline: 1.0218x; 1.0218x over previous
"""TRN2 Bass kernel for nn_Attention_87308095193383.

Sharding: 8 cores = (batch b in 0..3) x (query-half h in 0..1).
Host permutes N columns per core so "my queries" are columns 0:NQ.

Per core:
  A. conv1/conv2 (f32r) + GroupNorm (bn_stats on bf16 raw + manual combine,
     group reduce/broadcast via tiny indicator matmuls, affine on Pool).
  B. pe_attn^T = sigmoid(p2^T p1) via fp8 DoubleRow matmuls -> pa bf16.
  C. qkv (f32r). k/q written as fp8 quadrant tiles [32part,2dh,2hg,n] via
     host-permuted weight columns; v bf16 with interleaved ones column.
  E. qk via fp8 DoubleRow (2x), gating on DVE (some tiles via Act-copy +
     Pool-mult), exp on Act in 4096-wide ops, attn@v TRANSPOSED (out [q,65])
     in bf16 with 4 query-tile accumulators packed per PSUM bank.
  F. division via per-qt tensor_scalar, DMA-transpose o^T -> o_c, proj with
     bf16 moving operand, bias folded into Act Identity copy.
"""
import numpy as np
import ml_dtypes

import concourse.bass as bass
import concourse.mybir as mybir
import concourse.tile as tile
from concourse import bacc
from concourse.bass_utils import run_bass_kernel_spmd

F32R = mybir.dt.float32r
F32 = mybir.dt.float32
BF16 = mybir.dt.bfloat16
FP8 = mybir.dt.float8e4
AF = mybir.ActivationFunctionType
ALU = mybir.AluOpType
DR = mybir.MatmulPerfMode.DoubleRow

N_CORES = 8
C = 512          # channels
CT = C // 128    # 4 c-tiles
N = 2048         # sequence length
NT = N // 128    # 16 m-tiles
NQ = 1024        # queries per core
H = 8            # heads
D = 64           # head dim
SCALE = D ** -0.5
EPS = 1e-5

DEBUG = False
PEATTN_FP8 = False     # pe_attn matmul in fp8 DoubleRow
QK_FP8 = True         # q@k in fp8 DoubleRow
POOL_GATE = (0,)      # which mtp in 0..3 gate via Act-copy + Pool-mult

PA_DT = FP8 if PEATTN_FP8 else BF16


def build():
    nc = bacc.Bacc("TRN2", target_bir_lowering=False, debug=False,
                   num_devices=N_CORES)

    def din(name, shape, dt=F32R):
        return nc.dram_tensor(name, shape, dt, kind="ExternalInput").ap()

    peT = din("peT", [C, N], BF16)
    xT = din("xT", [C, N], BF16)
    cw1 = din("cw1", [C, C], BF16)  # conv1_w.T  [c_in, o]
    cw2 = din("cw2", [C, C], BF16)
    qw = din("qw", [C, 3 * C], BF16)  # qkv_w.T, q/k sections col-permuted
    pw = din("pw", [C, C], BF16)    # proj_w.T (bf16: moving o_c is bf16)
    cb1 = din("cb1", [C], F32)
    cb2 = din("cb2", [C], F32)
    gn1g = din("gn1g", [C], F32)
    gn1b = din("gn1b", [C], F32)
    gn2g = din("gn2g", [C], F32)
    gn2b = din("gn2b", [C], F32)
    pb = din("pb", [C], F32)
    gmask_in = din("gmask", [128, 2], F32)
    gmaskT_in = din("gmaskT", [2, 128], F32)
    vones_in = din("vones", [128, NT * H], BF16)
    ident_in = din("ident", [128, 128], BF16)
    outT = nc.dram_tensor("outT", [C, NQ], F32, kind="ExternalOutput").ap()
    dbg = {}
    if DEBUG:
        def dout(name, shape, dt):
            dbg[name] = nc.dram_tensor("dbg_" + name, shape, dt,
                                       kind="ExternalOutput").ap()
        dout("p1r", [128, CT, N], BF16)
        dout("p2r", [128, CT, N], BF16)
        dout("p1", [128, CT, NQ], PA_DT)
        dout("p2", [128, CT, N], PA_DT)
        dout("pa", [128, NT, NQ], BF16)
        dout("k8", [128, 2, 2, N], FP8 if QK_FP8 else BF16)
        dout("q8", [128, 2, 2, NQ], FP8 if QK_FP8 else BF16)
        dout("v", [128, NT, H, D + 1], BF16)
        dout("oT", [128, 8, H, D], BF16)
        dout("oc", [128, 8, CT, 128], BF16)

    with tile.TileContext(nc) as tc:
        _body(nc, tc, peT, xT, cw1, cw2, qw, pw, cb1, cb2,
              gn1g, gn1b, gn2g, gn2b, pb, gmask_in, gmaskT_in, vones_in,
              ident_in, outT, dbg)
    nc.compile()
    return nc


def _body(nc, tc, peT, xT, cw1, cw2, qw, pw, cb1, cb2,
          gn1g, gn1b, gn2g, gn2b, pb, gmask_in, gmaskT_in, vones_in,
          ident_in, outT, dbg):
    from contextlib import ExitStack
    ctx = ExitStack()
    with ctx:
        consts = ctx.enter_context(tc.tile_pool(name="consts", bufs=1))
        work = ctx.enter_context(tc.tile_pool(name="work", bufs=3))


        # ---- persistent activations
        pa_pool = ctx.enter_context(tc.tile_pool(name="pa", bufs=1))
        pa = pa_pool.tile([128, NT, NQ], BF16)     # sigmoid gate ^T tiles

        p12_pool = ctx.enter_context(tc.tile_pool(name="p12", bufs=1))
        kqv_pool = ctx.enter_context(tc.tile_pool(name="kqv", bufs=1))
        x_pool = ctx.enter_context(tc.tile_pool(name="x_pool", bufs=1))
        qw_pool = ctx.enter_context(tc.tile_pool(name="qw_pool", bufs=1))
        ps_main = ctx.enter_context(tc.tile_pool(name="ps_main", bufs=3,
                                                 space="PSUM"))

        # ================= stage A: conv + groupnorm =================
        ab = ExitStack()
        pe_pool = ab.enter_context(tc.tile_pool(name="pe_pool", bufs=1))
        cw_pool = ab.enter_context(tc.tile_pool(name="cw_pool", bufs=1))
        praw_pool = ab.enter_context(tc.tile_pool(name="praw", bufs=1))
        ps_gn = ab.enter_context(tc.tile_pool(name="ps_gn", bufs=1,
                                              space="PSUM"))

        pe_sb = pe_pool.tile([128, CT, N], BF16)
        pe_r = peT.rearrange("(t p) n -> p t n", p=128)
        cw1_sb = cw_pool.tile([128, CT, C], BF16)
        cw1_r = cw1.rearrange("(t p) o -> p t o", p=128)
        cw2_sb = cw_pool.tile([128, CT, C], BF16)
        nc.gpsimd.dma_start(cw1_sb[:, :, 0:256], cw1_r[:, :, 0:256])
        nc.sync.dma_start(pe_sb[:, :, 0:512], pe_r[:, :, 0:512])
        nc.gpsimd.dma_start(cw1_sb[:, :, 256:512], cw1_r[:, :, 256:512])
        nc.scalar.dma_start(pe_sb[:, :, 512:1024], pe_r[:, :, 512:1024])
        nc.sync.dma_start(pe_sb[:, :, 1024:1536], pe_r[:, :, 1024:1536])
        nc.scalar.dma_start(cw2_sb, cw2.rearrange("(t p) o -> p t o", p=128))
        nc.gpsimd.dma_start(pe_sb[:, :, 1536:2048], pe_r[:, :, 1536:2048])
        # x/qw/pw loads start now, drain behind the conv inputs
        x_sb = x_pool.tile([128, CT, N], BF16)
        x_r = xT.rearrange("(t p) n -> p t n", p=128)
        qw_sb = qw_pool.tile([128, CT, 3 * C], BF16)
        qw_r = qw.rearrange("(t p) o -> p t o", p=128)
        for cc, eng in enumerate((nc.sync, nc.scalar, nc.gpsimd, nc.sync)):
            eng.dma_start(x_sb[:, :, cc * 512:(cc + 1) * 512],
                          x_r[:, :, cc * 512:(cc + 1) * 512])
        for sec, eng in ((2, nc.sync), (1, nc.scalar), (0, nc.gpsimd)):
            eng.dma_start(qw_sb[:, :, sec * C:(sec + 1) * C],
                          qw_r[:, :, sec * C:(sec + 1) * C])
        # ---- constants (emitted on scalar queue to keep sync free for pe)
        gmask = consts.tile([128, 2], F32)
        nc.gpsimd.dma_start(gmask, gmask_in)
        gmaskT = consts.tile([2, 128], F32)
        nc.gpsimd.dma_start(gmaskT, gmaskT_in)
        epst = consts.tile([128, 1], F32)
        nc.vector.memset(epst, EPS)
        bias1 = consts.tile([128, CT], F32)
        nc.sync.dma_start(bias1, cb1.rearrange("(t p) -> p t", p=128))
        bias2 = consts.tile([128, CT], F32)
        nc.sync.dma_start(bias2, cb2.rearrange("(t p) -> p t", p=128))
        g1g = consts.tile([128, CT], F32)
        nc.sync.dma_start(g1g, gn1g.rearrange("(t p) -> p t", p=128))
        g1b = consts.tile([128, CT], F32)
        nc.sync.dma_start(g1b, gn1b.rearrange("(t p) -> p t", p=128))
        g2g = consts.tile([128, CT], F32)
        nc.sync.dma_start(g2g, gn2g.rearrange("(t p) -> p t", p=128))
        g2b = consts.tile([128, CT], F32)
        nc.sync.dma_start(g2b, gn2b.rearrange("(t p) -> p t", p=128))
        pbias = consts.tile([128, CT], F32)
        nc.sync.dma_start(pbias, pb.rearrange("(t p) -> p t", p=128))
        ident = consts.tile([128, 128], BF16)
        nc.sync.dma_start(ident, ident_in)

        p1_raw = praw_pool.tile([128, CT, N], BF16)
        p2_raw = praw_pool.tile([128, CT, N], BF16)
        p1_sb = p12_pool.tile([128, CT, NQ], PA_DT)
        p2_sb = p12_pool.tile([128, CT, N], PA_DT)

        for cwsb, cbt, gg, gb, raw, dst, keep in (
                (cw1_sb, bias1, g1g, g1b, p1_raw, p1_sb, NQ),
                (cw2_sb, bias2, g2g, g2b, p2_raw, p2_sb, N)):
            # conv matmuls + biased copies
            for nc2 in range(2):
                for ot in range(CT):
                    T = ps_main.tile([128, 2, 512], F32, tag="mm")
                    for half in range(2):
                        nch = nc2 * 2 + half
                        for ct in range(CT):
                            nc.tensor.matmul(
                                T[:, half], cwsb[:, ct, ot * 128:(ot + 1) * 128],
                                pe_sb[:, ct, nch * 512:(nch + 1) * 512],
                                start=(ct == 0), stop=(ct == CT - 1))
                    nc.scalar.activation(
                        raw[:, ot, nc2 * 1024:(nc2 + 1) * 1024], T,
                        AF.Identity, bias=cbt[:, ot:ot + 1])
            # stats on biased bf16 raw
            st = work.tile([128, CT, 4, 6], F32, tag="gnstats")
            for ot in range(CT):
                for ch in range(4):
                    nc.vector.bn_stats(st[:, ot, ch],
                                       raw[:, ot, ch * 512:(ch + 1) * 512])
            # combine chunk stats -> per-channel [mean, E[x^2]]
            A = work.tile([128, CT, 4], F32, tag="gnA")
            nc.vector.tensor_add(A, st[:, :, :, 1], st[:, :, :, 4])
            G2 = work.tile([128, CT, 4], F32, tag="gnG2")
            nc.vector.tensor_add(G2, st[:, :, :, 2], st[:, :, :, 5])
            P = work.tile([128, CT, 4], F32, tag="gnP")
            nc.vector.tensor_mul(P, st[:, :, :, 1], st[:, :, :, 1])
            Q = work.tile([128, CT, 4], F32, tag="gnQ")
            nc.vector.tensor_mul(Q, st[:, :, :, 4], st[:, :, :, 4])
            nc.vector.tensor_add(P, P, Q)
            nc.vector.tensor_scalar_mul(P, P, 256.0)
            nc.vector.tensor_add(P, P, G2)           # h per chunk
            SA = work.tile([128, CT, 2], F32, tag="gnSA")
            nc.vector.tensor_add(SA, A[:, :, 0:2], A[:, :, 2:4])
            SH = work.tile([128, CT, 2], F32, tag="gnSH")
            nc.vector.tensor_add(SH, P[:, :, 0:2], P[:, :, 2:4])
            stack = work.tile([128, 2, CT], F32, tag="gnstack")
            La = work.tile([128, CT], F32, tag="gnL0")
            nc.vector.tensor_add(La, SA[:, :, 0], SA[:, :, 1])
            nc.vector.tensor_scalar_mul(stack[:, 0], La, 1.0 / 8.0)
            nc.vector.tensor_add(La, SH[:, :, 0], SH[:, :, 1])
            nc.vector.tensor_scalar_mul(stack[:, 1], La, 1.0 / 2048.0)
            # group sums over 64-partition halves
            gsp = ps_gn.tile([2, 2, CT], F32, tag="gn")
            nc.tensor.matmul(gsp, gmask, stack.rearrange("p a t -> p (a t)"),
                             start=True, stop=True)
            gss = work.tile([2, 2, CT], F32, tag="gss")
            nc.vector.tensor_copy(gss, gsp)
            msr = work.tile([2, 2, CT], F32, tag="gmsr")  # [mean, rstd]
            nc.vector.tensor_scalar_mul(msr[:, 0], gss[:, 0], 1.0 / 64.0)
            vt = work.tile([2, 2, CT], F32, tag="gvt")
            nc.vector.tensor_scalar_mul(vt[:, 0], gss[:, 1], 1.0 / 64.0)
            nc.vector.tensor_mul(vt[:, 1], msr[:, 0], msr[:, 0])
            nc.vector.tensor_sub(vt[:, 0], vt[:, 0], vt[:, 1])
            nc.scalar.activation(vt[:, 0], vt[:, 0], AF.Sqrt, bias=epst[0:2])
            nc.vector.reciprocal(msr[:, 1], vt[:, 0])
            # broadcast [mean, rstd] to 128 partitions
            bcp = ps_gn.tile([128, 2, CT], F32, tag="gnb")
            nc.tensor.matmul(bcp, gmaskT, msr.rearrange("p a t -> p (a t)"),
                             start=True, stop=True)
            bcst = work.tile([128, 2, CT], F32, tag="gbc")
            nc.vector.tensor_copy(bcst, bcp)
            sc = work.tile([128, 2, CT], F32, tag="gsc")
            nc.vector.tensor_mul(sc[:, 0], bcst[:, 1], gg)
            nc.vector.tensor_mul(sc[:, 1], bcst[:, 0], sc[:, 0])
            nc.vector.tensor_sub(sc[:, 1], gb, sc[:, 1])
            for ot in range(CT):
                eng = nc.gpsimd if ot < 2 else nc.vector
                eng.tensor_scalar(
                    dst[:, ot, 0:keep], raw[:, ot, 0:keep],
                    sc[:, 0, ot:ot + 1], sc[:, 1, ot:ot + 1],
                    op0=ALU.mult, op1=ALU.add)

        if dbg:
            nc.sync.dma_start(dbg["p1r"], p1_raw)
            nc.sync.dma_start(dbg["p2r"], p2_raw)
            nc.sync.dma_start(dbg["p1"], p1_sb)
            nc.sync.dma_start(dbg["p2"], p2_sb)
        ab.close()

        # ================= stage B: pa = sigmoid(p2^T p1) =================
        # ================= stage C: qkv =================
        KQ_DT = FP8 if QK_FP8 else BF16
        k8 = kqv_pool.tile([128, 2, 2, N], KQ_DT)    # [quad-part, dh, hg, m]
        q8 = kqv_pool.tile([128, 2, 2, NQ], KQ_DT)
        v_sb = kqv_pool.tile([128, NT, H, D + 1], BF16)
        nc.sync.dma_start(
            v_sb[:, :, :, D:D + 1].rearrange("p t o u -> p (t o u)"),
            vones_in)

        def emit_v(nts=range(NT)):
            for nt in nts:
                T = ps_main.tile([128, 2, 512], F32, tag="mm")
                for ct in range(CT):
                    nc.tensor.matmul(
                        T[:, 0], x_sb[:, ct, nt * 128:(nt + 1) * 128],
                        qw_sb[:, ct, 2 * C:2 * C + 512],
                        start=(ct == 0), stop=(ct == CT - 1))
                nc.scalar.copy(v_sb[:, nt, :, 0:D],
                               T[:, 0].rearrange("p (h d) -> p h d", h=H))

        def emit_kq(hg, dhs=(0, 1)):
            for dh in dhs:
                ti = dh * 2 + hg
                for nch in range(4):
                    T = ps_main.tile([128, 2, 512], F32, tag="mm")
                    for ct in range(CT):
                        nc.tensor.matmul(
                            T[:, 0],
                            qw_sb[:, ct, C + ti * 128:C + (ti + 1) * 128],
                            x_sb[:, ct, nch * 512:(nch + 1) * 512],
                            start=(ct == 0), stop=(ct == CT - 1))
                    keng = nc.vector if nch % 2 else nc.scalar
                    if keng is nc.vector:
                        keng.tensor_copy(
                            k8[:, dh, hg, nch * 512:(nch + 1) * 512], T[:, 0])
                    else:
                        keng.copy(
                            k8[:, dh, hg, nch * 512:(nch + 1) * 512], T[:, 0])
                for nch in range(2):
                    T = ps_main.tile([128, 2, 512], F32, tag="mm")
                    for ct in range(CT):
                        nc.tensor.matmul(
                            T[:, 0], qw_sb[:, ct, ti * 128:(ti + 1) * 128],
                            x_sb[:, ct, nch * 512:(nch + 1) * 512],
                            start=(ct == 0), stop=(ct == CT - 1))
                    if nch % 2:
                        nc.vector.tensor_copy(
                            q8[:, dh, hg, nch * 512:(nch + 1) * 512], T[:, 0])
                    else:
                        nc.scalar.copy(
                            q8[:, dh, hg, nch * 512:(nch + 1) * 512], T[:, 0])

        def emit_peattn(nqch, mtps=range(NT // 2)):
            # pa holds tanh(z/2); sigmoid = (1 + tanh)/2, the 1/2 folded
            # into the exp scale downstream.
            for mtp in mtps:
                T = ps_main.tile([128, 2, 512], F32, tag="mm")
                for j in range(2):
                    mt = 2 * mtp + j
                    if PEATTN_FP8:
                        for cp in range(2):
                            nc.tensor.matmul(
                                T[:, j],
                                p2_sb[:, 2 * cp:2 * cp + 2, mt * 128:(mt + 1) * 128],
                                p1_sb[:, 2 * cp:2 * cp + 2, nqch * 512:(nqch + 1) * 512],
                                start=(cp == 0), stop=(cp == 1), perf_mode=DR)
                    else:
                        for ct in range(CT):
                            nc.tensor.matmul(
                                T[:, j], p2_sb[:, ct, mt * 128:(mt + 1) * 128],
                                p1_sb[:, ct, nqch * 512:(nqch + 1) * 512],
                                start=(ct == 0), stop=(ct == CT - 1))
                pa_sl = pa[:, 2 * mtp:2 * mtp + 2,
                           nqch * 512:(nqch + 1) * 512]
                nc.scalar.activation(pa_sl, T, AF.Tanh, scale=0.5)
                nc.gpsimd.tensor_scalar_add(pa_sl, pa_sl, 1.0)

        if dbg:
            nc.sync.dma_start(dbg["pa"], pa)
            nc.sync.dma_start(dbg["k8"], k8)
            nc.sync.dma_start(dbg["q8"], q8)
            nc.sync.dma_start(dbg["v"], v_sb)

        # ================= stage E: attention =================
        e_pools = ExitStack()
        ps_av = e_pools.enter_context(tc.tile_pool(name="ps_av", bufs=2,
                                                   space="PSUM"))
        te_pool = e_pools.enter_context(tc.tile_pool(name="te", bufs=3))
        cp_pool = e_pools.enter_context(tc.tile_pool(name="cp", bufs=3))
        o_pool = e_pools.enter_context(tc.tile_pool(name="o", bufs=1))
        pw_pool = e_pools.enter_context(tc.tile_pool(name="pw", bufs=1))
        fin_pool = e_pools.enter_context(tc.tile_pool(name="fin", bufs=2))

        pw_sb = pw_pool.tile([128, CT, C], BF16)
        nc.sync.dma_start(pw_sb, pw.rearrange("(t p) o -> p t o", p=128))
        oT = o_pool.tile([128, 8, H, D], BF16)       # [q-part, qtg, h, d]
        o_c = o_pool.tile([128, 8, CT, 128], BF16)   # [q-part, qtg, ct, q128]
        outT_r = outT.rearrange("(t p) n -> p t n", p=128)

        def emit_qk_gate_exp(nq, h, half, par=0):
            """qk (fp8 DR), gating, exp for 8 m-tiles. Returns e2 tile."""
            hm4, hg = h % 4, h // 4
            p0 = hm4 * 32
            t2 = te_pool.tile([128, 8, 512], BF16, tag="t2")
            for mtp in range(4):
                S = ps_main.tile([128, 2, 512], F32, tag="mm")
                for j in range(2):
                    mt = half * 8 + 2 * mtp + j
                    if QK_FP8:
                        nc.tensor.matmul(
                            S[:, j],
                            k8[p0:p0 + 32, :, hg, mt * 128:(mt + 1) * 128],
                            q8[p0:p0 + 32, :, hg, nq * 512:(nq + 1) * 512],
                            start=True, stop=True, perf_mode=DR,
                            tile_position=(p0, 0))
                    else:
                        for dh in range(2):
                            nc.tensor.matmul(
                                S[:, j],
                                k8[p0:p0 + 32, dh, hg, mt * 128:(mt + 1) * 128],
                                q8[p0:p0 + 32, dh, hg, nq * 512:(nq + 1) * 512],
                                start=(dh == 0), stop=(dh == 1),
                                tile_position=(p0, 0))
                pa_sl = pa[:, half * 8 + 2 * mtp:half * 8 + 2 * mtp + 2,
                           nq * 512:(nq + 1) * 512]
                if mtp in POOL_GATE:
                    sc_ = cp_pool.tile([128, 2, 512], BF16, tag="sc")
                    if par:
                        nc.vector.tensor_copy(sc_, S)
                    else:
                        nc.scalar.copy(sc_, S)
                    nc.gpsimd.tensor_mul(t2[:, 2 * mtp:2 * mtp + 2], sc_, pa_sl)
                else:
                    nc.vector.tensor_mul(t2[:, 2 * mtp:2 * mtp + 2], S, pa_sl)
            e2 = te_pool.tile([128, 8, 512], BF16, tag="e2")
            nc.scalar.activation(e2[:, 0:4], t2[:, 0:4], AF.Exp,
                                 scale=SCALE * 0.5)
            nc.scalar.activation(e2[:, 4:8], t2[:, 4:8], AF.Exp,
                                 scale=SCALE * 0.5)
            return e2

        def emit_av(nq, h, half, e2, uacc):
            for mtp8 in range(8):
                mt = half * 8 + mtp8
                for qt in range(4):
                    nc.tensor.matmul(
                        uacc[:, qt], e2[:, mtp8, qt * 128:(qt + 1) * 128],
                        v_sb[:, mt, h, :],
                        start=(mt == 0 and qt == 0),
                        stop=(mt == NT - 1 and qt == 3),
                        skip_group_check=True)

        def emit_div(nq, h, uacc):
            rec = work.tile([128, 4, 1], F32, tag="rec")
            nc.vector.reciprocal(rec, uacc[:, :, D:D + 1])
            for qt in range(4):
                nc.vector.tensor_scalar_mul(
                    oT[:, nq * 4 + qt, h, :], uacc[:, qt, 0:D],
                    rec[:, qt])

        def emit_proj_step(nq, step):
            if step < 4:
                qtg = nq * 4 + step
                src_q = oT[:, qtg].rearrange("p h d -> p (h d)")
                TRf = ps_main.tile([128, 2, 512], F32, tag="mm")
                TR = TRf[:, 0, 0:256].bitcast(BF16).rearrange(
                    "p (t n) -> p t n", t=4)
                for tb in range(4):
                    nc.tensor.matmul(
                        TR[:, tb], src_q[:, tb * 128:(tb + 1) * 128], ident,
                        is_transpose=True, start=(tb == 0), stop=(tb == 3),
                        skip_group_check=True)
                nc.scalar.copy(o_c[:, qtg], TR)
            else:
                ot = step - 4
                Pf = ps_main.tile([128, 2, 512], F32, tag="mm")
                P = Pf[:, 0]
                for ct in range(CT):
                    nc.tensor.matmul(
                        P, pw_sb[:, ct, ot * 128:(ot + 1) * 128],
                        o_c[:, nq * 4:(nq + 1) * 4, ct, :],
                        start=(ct == 0), stop=(ct == CT - 1))
                fin = fin_pool.tile([128, 512], F32, tag="fin")
                nc.scalar.activation(fin, P, AF.Identity,
                                     bias=pbias[:, ot:ot + 1])
                nc.sync.dma_start(outT_r[:, ot, nq * 512:(nq + 1) * 512], fin)

        def emit_proj(nq):
            for step in range(8):
                emit_proj_step(nq, step)

        # software-pipelined emission: av lags one block; div lags two;
        # proj lags three. Keeps each engine queue free of long waits.
        blocks = [(nq, h, half) for nq in range(2) for h in range(H)
                  for half in range(2)]
        av_pending = None
        div_sched = {}
        proj_sched = {}
        uacc_cur = None

        def flush(i):
            if i in div_sched:
                emit_div(*div_sched.pop(i))
            if i in proj_sched:
                nqp, step = proj_sched.pop(i)
                emit_proj_step(nqp, step)

        emit_v()
        emit_peattn(0)
        emit_kq(0)

        hooks = {8: lambda: (emit_peattn(1), emit_kq(1))}
        for i, (nq, h, half) in enumerate(blocks):
            if i in hooks:
                hooks.pop(i)()
            if half == 0:
                uacc_cur = ps_av.tile([128, 4, D + 1], F32, tag="av")
            e2 = emit_qk_gate_exp(nq, h, half, par=i % 2)
            if av_pending is not None:
                emit_av(*av_pending)
            av_pending = (nq, h, half, e2, uacc_cur)
            if half == 1:
                div_sched[i + 2] = (nq, h, uacc_cur)
                if h == H - 1:
                    for step in range(8):
                        proj_sched[i + 3 + step] = (nq, step)
            flush(i)
        emit_av(*av_pending)
        for j in sorted(set(list(div_sched) + list(proj_sched))):
            flush(j)
        if dbg:
            nc.sync.dma_start(dbg["oT"], oT)
            nc.sync.dma_start(dbg["oc"], o_c)
        e_pools.close()


_NC_CACHE = {}


def _get_nc():
    if "nc" not in _NC_CACHE:
        _NC_CACHE["nc"] = build()
    return _NC_CACHE["nc"]


def _qk_perm():
    """Column permutation for q/k sections of qkv_w so PSUM partition
    layout matches the fp8 quadrant tiles [32, dh, hg]."""
    perm = np.empty(C, np.int64)
    for ti in range(4):
        dh, hg = ti // 2, ti % 2
        for p in range(128):
            hm4, p32 = p // 32, p % 32
            h = hg * 4 + hm4
            d = dh * 32 + p32
            perm[ti * 128 + p] = h * D + d
    return perm


def make_in_maps(x, pe, qkv_w, proj_w, proj_b, conv1_w, conv1_b, gn1_g, gn1_b,
                 conv2_w, conv2_b, gn2_g, gn2_b):
    f = np.float32
    qwt = np.asarray(qkv_w, f).T          # [c_in, 3C]
    perm = _qk_perm()
    qw_host = np.concatenate([qwt[:, 0:C][:, perm],
                              qwt[:, C:2 * C][:, perm],
                              qwt[:, 2 * C:3 * C]], axis=1)
    shared = {
        "cw1": np.ascontiguousarray(np.asarray(conv1_w, f).T).astype(ml_dtypes.bfloat16),
        "cw2": np.ascontiguousarray(np.asarray(conv2_w, f).T).astype(ml_dtypes.bfloat16),
        "qw": np.ascontiguousarray(qw_host).astype(ml_dtypes.bfloat16),
        "pw": np.ascontiguousarray(np.asarray(proj_w, f).T).astype(ml_dtypes.bfloat16),
        "cb1": np.asarray(conv1_b, f),
        "cb2": np.asarray(conv2_b, f),
        "gn1g": np.asarray(gn1_g, f),
        "gn1b": np.asarray(gn1_b, f),
        "gn2g": np.asarray(gn2_g, f),
        "gn2b": np.asarray(gn2_b, f),
        "pb": np.asarray(proj_b, f),
        "gmask": np.repeat(np.eye(2, dtype=f), 64, axis=0),
        "gmaskT": np.ascontiguousarray(
            np.repeat(np.eye(2, dtype=f), 64, axis=0).T),
        "vones": np.ones((128, NT * H), np.float32).astype(ml_dtypes.bfloat16),
        "ident": np.eye(128, dtype=np.float32).astype(ml_dtypes.bfloat16),
    }
    in_maps = []
    for c in range(N_CORES):
        b, h = c // 2, c % 2
        xT = np.asarray(x[b], f).T
        peT = np.asarray(pe[b], f).T
        if h == 1:
            xT = np.concatenate([xT[:, NQ:], xT[:, :NQ]], axis=1)
            peT = np.concatenate([peT[:, NQ:], peT[:, :NQ]], axis=1)
        m = dict(shared)
        m["xT"] = np.ascontiguousarray(xT).astype(ml_dtypes.bfloat16)
        m["peT"] = np.ascontiguousarray(peT).astype(ml_dtypes.bfloat16)
        in_maps.append(m)
    return in_maps


def assemble_out(results):
    B = N_CORES // 2
    out = np.empty((B, N, C), np.float32)
    for c in range(N_CORES):
        b, h = c // 2, c % 2
        out[b, h * NQ:(h + 1) * NQ, :] = results[c]["outT"].T
    return out


def kernel(**inputs):
    nc = _get_nc()
    in_maps = make_in_maps(**inputs)
    r = run_bass_kernel_spmd(nc, in_maps, core_ids=list(range(N_CORES)))
    return assemble_out(r.results)


if __name__ == "__main__":
    nc = build()
    print("build+compile OK")
